# revision 21
# baseline (speedup 1.0000x reference)
"""AcceleratedInnerShiftTriple kernel for 8 TRN2 NeuronCores.

Reference math (B=4, C=512, H=W=64, N=4096, C2=256):
  former, latter = x[:, :256], x[:, 256:]   (each (B, 256, N) after reshape)
  flag[n] = mask[n] >= 1
  cos[b,n,m] = <latter_n/|latter_n|, latter_m/|latter_m|>, masked candidates m
  excluded (-inf); nn = argmax_m; shift = former[:, :, nn] where flag else 0
  out = concat([former, latter, shift], channel) -> (B, 768, 64, 64)

Key reductions used here:
  * out[:, :512] == x verbatim; only `shift` needs computing.
  * Query-side normalization is a positive per-row scale -> argmax-invariant.
    Only candidates need scaling; folded into the candidate matrix host-side.
  * Only masked positions are queries; only unmasked positions are candidates.
    Both sets are compacted host-side from the runtime mask (matmul shrinks
    from N x N to nq x nc), order-preserving so argmax tie semantics match.

Device work per core (2 cores per batch, half the queries each):
  scores = qT @ c_scaled via PE (float32r), argmax via DVE max8+find_index8,
  gather of former rows via GPSIMD indirect DMA, DMA out.
"""

import numpy as np

EPS = 1e-8
P = 128
CHUNK = 1024  # candidate chunk width (2 PSUM banks)
NEG = -1e30

# test.py toggles these for profiling
TRACE = False
LAST_EXEC_NS = None
LAST_RESULTS = None
LAST_TRACE = None
LAST_PROFILE_JSON = None
MATMUL_DTYPE = "float32r"  # or "float32"


def _install_profiling():
    """Register the NTFF profile hook that this container's antenv lacks.

    Best-effort: profiling is test-only; kernel correctness never depends
    on it.
    """
    import sys
    import types

    try:
        from antenv.axon_hooks import get_axon_ntff_profile_hook  # noqa: F401

        return True
    except ImportError:
        pass
    try:
        import antenv
        from trn_agent_boot.trn_boot import _ntff_profile_via_ctypes

        mod = types.ModuleType("antenv.axon_hooks")
        state = {}
        mod.set_axon_ntff_profile_hook = lambda h: state.update(hook=h)
        mod.get_axon_ntff_profile_hook = lambda: state.get("hook")
        sys.modules["antenv.axon_hooks"] = mod
        antenv.axon_hooks = mod
        mod.set_axon_ntff_profile_hook(
            _ntff_profile_via_ctypes("/opt/axon/libaxon_pjrt.so")
        )
        from concourse import bass_utils

        bass_utils.upload_artifacts = lambda tmpdir: tmpdir  # no S3 here
        return True
    except Exception as e:  # pragma: no cover
        print(f"profiling hook install failed: {e}")
        return False


def _build(nqp, ncp, kdim, ncand=None):
    """Build the SPMD Bass graph for one core: nqp queries x ncp candidates.

    ncand: number of real (non-pad) candidates; pad score columns are forced
    to NEG so a zero-feature pad column can never win the argmax.
    """
    import concourse.bass as bass
    import concourse.mybir as mybir
    from concourse.bacc import Bacc
    from concourse.tile import TileContext

    f32 = mybir.dt.float32
    mm_dt = getattr(mybir.dt, MATMUL_DTYPE)
    u32 = mybir.dt.uint32
    i32 = mybir.dt.int32
    # fp32r operands must be produced as fp32r (walrus checkMatmultFP32r);
    # typing the SBUF staging tiles as fp32r makes the in-DMA the rounding
    # producer
    in_dt = mm_dt

    assert nqp % P == 0 and ncp % CHUNK == 0 and kdim == 256
    nqb = nqp // P
    nch = ncp // CHUNK
    if ncand is None:
        ncand = ncp

    nc = Bacc()
    # queries and candidates packed in one partition-major tensor so a
    # single DMA (= single semaphore) produces every matmul input: the
    # fused fp32r LDW+MM has one sync-wait slot, multi-wait fails codegen
    w = nqp + ncp
    qc_ext = nc.declare_dram_parameter("qc", [P, 2, w], in_dt, isOutput=False)
    f_ext = nc.declare_dram_parameter("f", [ncp, kdim], f32, isOutput=False)
    out_ext = nc.declare_dram_parameter("out", [nqp, kdim], f32, isOutput=True)
    idx_ext = nc.declare_dram_parameter("outidx", [nqp, 1], i32, isOutput=True)

    with TileContext(nc) as tc:
        with (
            tc.tile_pool(name="persist", bufs=1) as persist,
            tc.tile_pool(name="blk", bufs=2) as blk,
            tc.tile_pool(name="small", bufs=2) as small,
            tc.tile_pool(name="psum", bufs=3, space="PSUM") as psum_pool,
        ):
            qc_sb = persist.tile([P, 2, w], in_dt)
            nc.sync.dma_start(out=qc_sb[:], in_=qc_ext[:])

            def q_ap(kc, qb):
                return qc_sb[:, kc, qb * P : (qb + 1) * P]

            def c_ap(kc, lo, hi):
                return qc_sb[:, kc, nqp + lo : nqp + hi]

            for qb in range(nqb):
                best_val = blk.tile([P, 1], f32, tag="bestv")
                best_idx = blk.tile([P, 1], f32, tag="besti")
                for ch in range(nch):
                    c0 = ch * CHUNK
                    ps = psum_pool.tile([P, CHUNK], f32, tag="scores")
                    for sub in range(0, CHUNK, 512):
                        for kc in range(2):
                            nc.tensor.matmul(
                                out=ps[:, sub : sub + 512],
                                lhsT=q_ap(kc, qb),
                                rhs=c_ap(kc, c0 + sub, c0 + sub + 512),
                                start=(kc == 0),
                                stop=(kc == 1),
                            )
                    if ncand < c0 + CHUNK:
                        pad0 = max(0, ncand - c0)
                        nc.vector.memset(ps[:, pad0:], NEG)
                    mx8 = small.tile([P, 8], f32, tag="mx8")
                    ix8 = small.tile([P, 8], u32, tag="ix8")
                    nc.vector.max(out=mx8[:], in_=ps[:])
                    nc.vector.max_index(out=ix8[:], in_max=mx8[:], in_values=ps[:])
                    ixf = small.tile([P, 1], f32, tag="ixf")
                    nc.vector.tensor_copy(out=ixf[:], in_=ix8[:, 0:1])
                    if ch == 0:
                        nc.vector.tensor_copy(out=best_val[:], in_=mx8[:, 0:1])
                        nc.vector.tensor_copy(out=best_idx[:], in_=ixf[:])
                    else:
                        nc.vector.tensor_scalar(
                            ixf[:], ixf[:], float(c0), scalar2=None,
                            op0=mybir.AluOpType.add,
                        )
                        gt = small.tile([P, 1], u32, tag="gt")
                        nc.vector.tensor_tensor(
                            out=gt[:], in0=mx8[:, 0:1], in1=best_val[:],
                            op=mybir.AluOpType.is_gt,
                        )
                        nc.vector.copy_predicated(best_val[:], gt[:], mx8[:, 0:1])
                        nc.vector.copy_predicated(best_idx[:], gt[:], ixf[:])

                idx_i = blk.tile([P, 1], i32, tag="idxi")
                nc.vector.tensor_copy(out=idx_i[:], in_=best_idx[:])
                g = blk.tile([P, kdim], f32, tag="gath")
                nc.gpsimd.indirect_dma_start(
                    out=g[:],
                    out_offset=None,
                    in_=f_ext[:],
                    in_offset=bass.IndirectOffsetOnAxis(ap=idx_i[:, :1], axis=0),
                )
                nc.sync.dma_start(out=out_ext[qb * P : (qb + 1) * P, :], in_=g[:])
                nc.sync.dma_start(out=idx_ext[qb * P : (qb + 1) * P, :], in_=idx_i[:])
    if not nc.is_finalized():
        nc.finalize()
    return nc


def _ceil_to(v, m):
    return max(m, ((v + m - 1) // m) * m)


def kernel(x, mask):
    global LAST_EXEC_NS, LAST_RESULTS
    x = np.ascontiguousarray(np.asarray(x, dtype=np.float32))
    mask = np.asarray(mask, dtype=np.float32)
    B, C, H, W = x.shape
    C2 = C // 2
    N = H * W
    former = x[:, :C2].reshape(B, C2, N)
    latter = x[:, C2:].reshape(B, C2, N)
    flag = mask.reshape(N) >= 1.0
    qs = np.flatnonzero(flag)
    cs = np.flatnonzero(~flag)
    nq, ncand = len(qs), len(cs)

    shift = np.zeros((B, C2, N), np.float32)
    if nq > 0 and ncand == 0:
        # all candidates masked: argmax of all -inf rows is 0
        shift[:, :, qs] = former[:, :, 0][:, :, None]
    elif nq > 0:
        assert B * 2 == 8, "sharding hardcoded for B=4 over 8 cores"
        h = (nq + 1) // 2
        halves = [qs[:h], qs[h:]]
        nqp = _ceil_to(h, P)
        ncp = _ceil_to(ncand, CHUNK)

        # host prep: candidate scaling (positive scale per candidate column)
        cf = latter[:, :, cs]  # (B, 256, ncand)
        nrm = np.linalg.norm(cf, axis=1)  # f32, matches reference dtype
        c_scaled = cf * (1.0 / (nrm + EPS))[:, None, :]

        in_maps = []
        for core in range(8):
            b, hi = divmod(core, 2)
            qh = halves[hi]
            qc = np.zeros((P, 2, nqp + ncp), np.float32)
            if len(qh):
                qc[:, :, : len(qh)] = (
                    latter[b][:, qh].reshape(2, P, len(qh)).transpose(1, 0, 2)
                )
            qc[:, :, nqp : nqp + ncand] = (
                c_scaled[b].reshape(2, P, ncand).transpose(1, 0, 2)
            )
            f = np.zeros((ncp, C2), np.float32)
            f[:ncand] = former[b][:, cs].T
            in_maps.append({"qc": qc, "f": f})

        from concourse.bass_utils import run_bass_kernel_spmd

        trace = TRACE and _install_profiling()
        nc = _build(nqp, ncp, C2, ncand=ncand)
        res = run_bass_kernel_spmd(nc, in_maps, core_ids=list(range(8)), trace=trace)
        LAST_EXEC_NS = res.exec_time_ns
        LAST_RESULTS = res.results
        global LAST_TRACE, LAST_PROFILE_JSON
        if res.instructions_and_trace is not None:
            LAST_TRACE = res.instructions_and_trace[1]
        LAST_PROFILE_JSON = res.profile_json
        # masked-candidate exclusion pads score rows with NEG via zero
        # features; zero-feature pad queries are discarded here
        for core in range(8):
            b, hi = divmod(core, 2)
            qh = halves[hi]
            if len(qh):
                shift[b][:, qh] = res.results[core]["out"][: len(qh)].T

    out = np.concatenate([former, latter, shift], axis=1)
    return out.reshape(B, 3 * C2, H, W)
